# revision 1
# baseline (speedup 1.0000x reference)
"""DINN forward kernel for Trainium2 (Bass/Tile), batch-sharded across 8 NeuronCores.

Reference computation (B=16384, D=512):
    gates  = sigmoid(x @ W.T + b)                       # [B, D]
    linear = sum(gates * x, axis=1)                     # [B]
    quad   = sum_{i<j} iw_ij * x_i * x_j                # [B]
    out    = sigmoid(linear + quad)[:, None]            # [B, 1]

Data-parallel sharding: x is split along the batch across the 8 cores;
W, b and the (strictly upper-triangular) interaction matrix U built from iw
are replicated. No collectives are needed in the forward pass.

Per-core kernel (batch shard of 2048 rows), all matmuls in the "transposed"
orientation with the contraction dim D on SBUF partitions (host pre-transposes
x -> xT so no on-chip transposes are needed):
    G^T[dout, b] = sum_k Wt[k, dout] xT[k, b]   (f32r matmuls, 1 cy/row)
    T^T[dout, b] = sum_k U[k, dout] xT[k, b]    (fp32 exact; U strictly upper
                                                 -> the 6/16 lower blocks are
                                                 skipped entirely)
    sig = sigmoid(G^T + b)     on ACT (per-partition bias)
    P   = (sig + T^T) * xT     on DVE (two grouped 4-bank ops per batch tile)
    q   = ones^T @ P           fp32 PE matmul reduces P over partitions
The kernel returns the log-odds; the final sigmoid is applied on host in
float64 (exact, and the kernel's output stays well-conditioned).

Precision (hardware-measured): float32r matmuls carry ~13-bit mantissas (the
f32r DMA itself rounds), fine for the gates (log-odds error ~1e-3) but not for
the quad term, whose |log-odds| ~ 360 scale needs exact fp32 products. This
mix gives max abs output error ~4e-4 vs the fp32 reference.
"""
import sys

if "/opt/trn_rl_repo" not in sys.path:
    sys.path.insert(0, "/opt/trn_rl_repo")

import numpy as np

import concourse.tile as tile
from concourse import bacc, bass_isa, mybir
from concourse.bass_utils import run_bass_kernel_spmd

B, D = 16384, 512
NCORES = 8
BC = B // NCORES            # 2048 rows per core
NBT = BC // 512             # 4 batch tiles per core
NK = D // 128               # 4 contraction chunks

f32 = mybir.dt.float32
f32r = mybir.dt.float32r
AF = mybir.ActivationFunctionType

_CACHE = {}


def _build():
    nc = bacc.Bacc("TRN2", target_bir_lowering=False, debug=False,
                   num_devices=NCORES)

    d_xTf = nc.declare_dram_parameter("xTf", [D, BC], f32, isOutput=False)
    d_Wtr = nc.declare_dram_parameter("Wtr", [D, D], f32r, isOutput=False)
    d_Uf = nc.declare_dram_parameter("Uf", [D, D], f32, isOutput=False)
    d_bias = nc.declare_dram_parameter("bias", [D], f32, isOutput=False)
    d_out = nc.declare_dram_parameter("out", [1, BC], f32, isOutput=True)

    rearr = lambda ap: ap.rearrange("(c p) n -> p c n", p=128)

    with tile.TileContext(nc) as tc:
        with tc.tile_pool(name="const", bufs=1) as const, \
             tc.tile_pool(name="xin", bufs=2) as xin, \
             tc.tile_pool(name="elt", bufs=2) as elt, \
             tc.tile_pool(name="pg", bufs=3, space="PSUM") as pg, \
             tc.tile_pool(name="pt", bufs=1, space="PSUM") as pt:

            # ---- weights / constants (loaded once, replicated per core) ----
            Wtr_sb = const.tile([128, NK, D], f32r, tag="wtr")
            Uf_sb = const.tile([128, NK, D], f32, tag="uf")
            bias_sb = const.tile([128, NK], f32, tag="bias")
            out_sb = const.tile([1, BC], f32, tag="out_sb")

            xTf_r = rearr(d_xTf[:, :])
            # f32r view of the same fp32 dram tensor (no duplicate upload);
            # the f32r DMA rounds, which the gates tolerate.
            xTr_r = rearr(d_xTf[:, :].bitcast(f32r))
            Wtr_d = rearr(d_Wtr[:, :])

            # first transfers in need order, gates k=0 pieces first, so the
            # PE starts ~10us earlier instead of waiting for whole tensors
            xf0 = xin.tile([128, NK, 512], f32, tag="xf")
            xr0 = xin.tile([128, NK, 512], f32r, tag="xr")
            Uf_d = rearr(d_Uf[:, :])
            nc.sync.dma_start(out=xr0[:, 0, :], in_=xTr_r[:, 0, 0:512])
            nc.sync.dma_start(out=Wtr_sb[:, 0, :], in_=Wtr_d[:, 0, :])
            nc.sync.dma_start(out=xr0[:, 1:NK, :], in_=xTr_r[:, 1:NK, 0:512])
            nc.sync.dma_start(out=Wtr_sb[:, 1:NK, :], in_=Wtr_d[:, 1:NK, :])
            nc.sync.dma_start(out=xf0[:, 0, :], in_=xTf_r[:, 0, 0:512])
            nc.sync.dma_start(out=Uf_sb[:, 0, :], in_=Uf_d[:, 0, :])
            nc.sync.dma_start(
                out=bias_sb, in_=d_bias[:].rearrange("(c p) -> p c", p=128))
            nc.sync.dma_start(out=xf0[:, 1:NK, :], in_=xTf_r[:, 1:NK, 0:512])
            nc.sync.dma_start(out=Uf_sb[:, 1:NK, :], in_=Uf_d[:, 1:NK, :])

            for b0 in range(NBT):
                bsl = slice(b0 * 512, (b0 + 1) * 512)
                if b0 == 0:
                    xf, xr = xf0, xr0
                else:
                    xf = xin.tile([128, NK, 512], f32, tag="xf")
                    xr = xin.tile([128, NK, 512], f32r, tag="xr")
                    nc.sync.dma_start(out=xr, in_=xTr_r[:, :, bsl])
                    nc.sync.dma_start(out=xf, in_=xTf_r[:, :, bsl])

                pt_all = pt.tile([128, NK, 512], f32, tag="pt")   # 4 banks
                sig_all = elt.tile([128, NK, 512], f32, tag="sig")

                for m in range(NK):
                    msl = slice(m * 128, (m + 1) * 128)
                    # gates: G^T chunk, f32r (1 cy/row)
                    psum_g = pg.tile([128, 512], f32, tag="pg")
                    for k in range(NK):
                        nc.tensor.matmul(
                            psum_g, Wtr_sb[:, k, msl], xr[:, k, :],
                            start=(k == 0), stop=(k == NK - 1))

                    # quad: T^T chunk, exact fp32; skip zero blocks (k > m)
                    tdst = pt_all[:, m, :]
                    for k in range(m + 1):
                        nc.tensor.matmul(
                            tdst, Uf_sb[:, k, msl], xf[:, k, :],
                            start=(k == 0), stop=(k == m))

                    nc.scalar.activation(sig_all[:, m, :], psum_g, AF.Sigmoid,
                                         bias=bias_sb[:, m:m + 1], scale=1.0)

                # P = (sig + T^T) * xT.  For the last batch tile, split per
                # m-chunk so the DVE drain overlaps the final matmuls; for the
                # others the two grouped ops are cheaper (less per-op overhead)
                if b0 == NBT - 1:
                    p_all = elt.tile([128, NK, 512], f32, tag="p")
                    for m in range(NK):
                        s2m = elt.tile([128, 512], f32, tag="s2s")
                        nc.vector.tensor_add(s2m, sig_all[:, m, :],
                                             pt_all[:, m, :])
                        # multiply on GpSimd: pipelines against the DVE adds
                        # in the final tile's drain (operands are SBUF-only)
                        nc.gpsimd.tensor_mul(p_all[:, m, :], s2m, xf[:, m, :])
                else:
                    s2 = elt.tile([128, NK, 512], f32, tag="s2")
                    nc.vector.tensor_add(s2, sig_all, pt_all)
                    p_all = elt.tile([128, NK, 512], f32, tag="p")
                    nc.vector.tensor_mul(p_all, s2, xf)

                # reduce P over partitions on the (otherwise idle) GpSimd
                # engine: 2 DVE adds fold the 4 m-chunks, then one
                # partition_all_reduce; keeps all 16 ones-matmuls off the PE
                s4 = elt.tile([128, 2, 512], f32, tag="s4")
                nc.vector.tensor_add(s4, p_all[:, 0:2, :], p_all[:, 2:4, :])
                sred = elt.tile([128, 512], f32, tag="sred")
                nc.vector.tensor_add(sred, s4[:, 0, :], s4[:, 1, :])
                par = elt.tile([128, 512], f32, tag="par")
                nc.gpsimd.partition_all_reduce(par, sred, 128,
                                               bass_isa.ReduceOp.add)
                # evacuate log-odds (final sigmoid happens on host, exact);
                # last tile: DMA straight from par row 0, skipping the ACT
                if b0 == NBT - 1:
                    nc.sync.dma_start(out=d_out[0:1, bsl], in_=par[0:1, :])
                else:
                    nc.scalar.activation(out_sb[0:1, bsl], par[0:1, :],
                                         AF.Identity)

            nc.sync.dma_start(out=d_out[0:1, 0:(NBT - 1) * 512],
                              in_=out_sb[0:1, 0:(NBT - 1) * 512])

    nc.compile()
    return nc


def kernel(x, W, b, iw):
    x = np.asarray(x, np.float32)
    W = np.asarray(W, np.float32)
    b = np.asarray(b, np.float32)
    iw = np.asarray(iw, np.float32)

    # host prep: strictly upper-triangular U from iw (row-major i<j order),
    # pre-transposed operands so the contraction dim lands on SBUF partitions
    U = np.zeros((D, D), np.float32)
    iu, ju = np.triu_indices(D, k=1)
    U[iu, ju] = iw
    Wt = np.ascontiguousarray(W.T)          # lhsT for gates: Wt[k, dout]
    xT = x.T                                 # [D, B] view

    shared = {"Wtr": Wt, "Uf": U, "bias": b}
    in_maps = []
    for c in range(NCORES):
        m = dict(shared)
        m["xTf"] = np.ascontiguousarray(xT[:, c * BC:(c + 1) * BC])
        in_maps.append(m)

    if "nc" not in _CACHE:
        _CACHE["nc"] = _build()
    nc = _CACHE["nc"]

    res = run_bass_kernel_spmd(nc, in_maps, list(range(NCORES)))
    lo = np.concatenate(
        [res.results[c]["out"][0] for c in range(NCORES)]).astype(np.float64)
    out = 1.0 / (1.0 + np.exp(-np.clip(lo, -708.0, 708.0)))
    return out.reshape(B, 1).astype(np.float32)



# revision 2
# speedup vs baseline: 1.1691x; 1.1691x over previous
"""DINN forward kernel for Trainium2 (Bass/Tile), batch-sharded across 8 NeuronCores.

Reference computation (B=16384, D=512):
    gates  = sigmoid(x @ W.T + b)                       # [B, D]
    linear = sum(gates * x, axis=1)                     # [B]
    quad   = sum_{i<j} iw_ij * x_i * x_j                # [B]
    out    = sigmoid(linear + quad)[:, None]            # [B, 1]

Data-parallel sharding: x is split along the batch across the 8 cores;
W, b and the (strictly upper-triangular) interaction matrix U built from iw
are replicated. No collectives are needed in the forward pass.

Per-core kernel (batch shard of 2048 rows), all matmuls in the "transposed"
orientation with the contraction dim D on SBUF partitions (host pre-transposes
x -> xT so no on-chip transposes are needed). Everything runs in f32r
(~13-bit mantissa, 1 PE cycle/row vs 4 for exact fp32):
    G^T[dout, b] = sum_k Wt[k, dout] xT[k, b]
    T^T[dout, b] = sum_k U[k, dout] xT[k, b]   (U strictly upper -> the 6/16
                                                lower blocks are skipped)
    sig = sigmoid(G^T + b)     on ACT (per-partition bias)
    s2  = sig + T^T            on DVE (grouped 4-bank op)
    P   = s2 * xT              on GpSimd/Pool (keeps DVE under the PE shadow)
    r   = partition-reduce(fold(P))  2 DVE folds + gpsimd partition_all_reduce
The kernel returns the log-odds; the final sigmoid is applied on host in
float64 (exact, and the kernel's output stays well-conditioned).

Precision: f32r carries ~13-bit mantissas (the f32r DMA itself rounds).
Quad log-odds error std ~ 1.2e-4 * sqrt(130816) ~ 0.04 on a +-360-scale
log-odds, which perturbs the final sigmoid only for the ~1% of rows near the
decision boundary: measured rel err stays ~1e-3, far under the 2e-2 gate.
x is DMA'd once as f32r and bitcast back to f32 for the element-wise ops.
"""
import sys

if "/opt/trn_rl_repo" not in sys.path:
    sys.path.insert(0, "/opt/trn_rl_repo")

import numpy as np

import concourse.tile as tile
from concourse import bacc, bass_isa, mybir
from concourse.bass_utils import run_bass_kernel_spmd

B, D = 16384, 512
NCORES = 8
BC = B // NCORES            # 2048 rows per core
NBT = BC // 512             # 4 batch tiles per core
NK = D // 128               # 4 contraction chunks

f32 = mybir.dt.float32
f32r = mybir.dt.float32r
AF = mybir.ActivationFunctionType

_CACHE = {}


def _build():
    nc = bacc.Bacc("TRN2", target_bir_lowering=False, debug=False,
                   num_devices=NCORES)

    d_xTf = nc.declare_dram_parameter("xTf", [D, BC], f32, isOutput=False)
    d_Wtr = nc.declare_dram_parameter("Wtr", [D, D], f32r, isOutput=False)
    d_Uf = nc.declare_dram_parameter("Uf", [D, D], f32r, isOutput=False)
    d_bias = nc.declare_dram_parameter("bias", [D], f32, isOutput=False)
    d_out = nc.declare_dram_parameter("out", [1, BC], f32, isOutput=True)

    rearr = lambda ap: ap.rearrange("(c p) n -> p c n", p=128)

    with tile.TileContext(nc) as tc:
        with tc.tile_pool(name="const", bufs=1) as const, \
             tc.tile_pool(name="xin", bufs=2) as xin, \
             tc.tile_pool(name="elt", bufs=2) as elt, \
             tc.tile_pool(name="pg", bufs=3, space="PSUM") as pg, \
             tc.tile_pool(name="pt", bufs=1, space="PSUM") as pt:

            # ---- weights / constants (loaded once, replicated per core) ----
            Wtr_sb = const.tile([128, NK, D], f32r, tag="wtr")
            Uf_sb = const.tile([128, NK, D], f32r, tag="uf")
            bias_sb = const.tile([128, NK], f32, tag="bias")
            out_sb = const.tile([1, BC], f32, tag="out_sb")

            # f32r view of the fp32 dram tensor (the f32r DMA rounds, which
            # all consumers tolerate; see module docstring)
            xTr_r = rearr(d_xTf[:, :].bitcast(f32r))
            Wtr_d = rearr(d_Wtr[:, :])
            Uf_d = rearr(d_Uf[:, :])

            # first transfers in need order, gates k=0 pieces first, so the
            # PE starts ~2us in instead of waiting for whole tensors
            xr0 = xin.tile([128, NK, 512], f32r, tag="xr")
            nc.sync.dma_start(out=xr0[:, 0, :], in_=xTr_r[:, 0, 0:512])
            nc.sync.dma_start(out=Wtr_sb[:, 0, :], in_=Wtr_d[:, 0, :])
            nc.sync.dma_start(out=xr0[:, 1:NK, :], in_=xTr_r[:, 1:NK, 0:512])
            nc.sync.dma_start(out=Wtr_sb[:, 1:NK, :], in_=Wtr_d[:, 1:NK, :])
            nc.sync.dma_start(out=Uf_sb[:, 0, :], in_=Uf_d[:, 0, :])
            nc.sync.dma_start(
                out=bias_sb, in_=d_bias[:].rearrange("(c p) -> p c", p=128))
            nc.sync.dma_start(out=Uf_sb[:, 1:NK, :], in_=Uf_d[:, 1:NK, :])

            for b0 in range(NBT):
                bsl = slice(b0 * 512, (b0 + 1) * 512)
                if b0 == 0:
                    xr = xr0
                else:
                    xr = xin.tile([128, NK, 512], f32r, tag="xr")
                    nc.sync.dma_start(out=xr, in_=xTr_r[:, :, bsl])
                xf = xr[:, :, :].bitcast(f32)

                pt_all = pt.tile([128, NK, 512], f32, tag="pt")   # 4 banks
                sig_all = elt.tile([128, NK, 512], f32, tag="sig")

                for m in range(NK):
                    msl = slice(m * 128, (m + 1) * 128)
                    # gates: G^T chunk
                    psum_g = pg.tile([128, 512], f32, tag="pg")
                    for k in range(NK):
                        nc.tensor.matmul(
                            psum_g, Wtr_sb[:, k, msl], xr[:, k, :],
                            start=(k == 0), stop=(k == NK - 1))

                    # quad: T^T chunk; skip zero blocks (k > m)
                    tdst = pt_all[:, m, :]
                    for k in range(m + 1):
                        nc.tensor.matmul(
                            tdst, Uf_sb[:, k, msl], xr[:, k, :],
                            start=(k == 0), stop=(k == m))

                    nc.scalar.activation(sig_all[:, m, :], psum_g, AF.Sigmoid,
                                         bias=bias_sb[:, m:m + 1], scale=1.0)

                # P = (sig + T^T) * xT.  For the last batch tile, split per
                # m-chunk so the drain overlaps the final matmuls; for the
                # others the grouped ops are cheaper (less per-op overhead)
                p_all = elt.tile([128, NK, 512], f32, tag="p")
                if b0 == NBT - 1:
                    for m in range(NK):
                        s2m = elt.tile([128, 512], f32, tag="s2s")
                        nc.vector.tensor_add(s2m, sig_all[:, m, :],
                                             pt_all[:, m, :])
                        nc.gpsimd.tensor_mul(p_all[:, m, :], s2m, xf[:, m, :])
                else:
                    s2 = elt.tile([128, NK, 512], f32, tag="s2")
                    nc.vector.tensor_add(s2, sig_all, pt_all)
                    # multiply on GpSimd/Pool: keeps the DVE under the PE
                    # shadow (operands are SBUF-only)
                    nc.gpsimd.tensor_mul(p_all, s2, xf)

                # reduce P over the 4 m-chunks (DVE folds), then over the 128
                # partitions on the (otherwise idle) GpSimd engine
                s4 = elt.tile([128, 2, 512], f32, tag="s4")
                nc.vector.tensor_add(s4, p_all[:, 0:2, :], p_all[:, 2:4, :])
                sred = elt.tile([128, 512], f32, tag="sred")
                nc.vector.tensor_add(sred, s4[:, 0, :], s4[:, 1, :])
                par = elt.tile([128, 512], f32, tag="par")
                nc.gpsimd.partition_all_reduce(par, sred, 128,
                                               bass_isa.ReduceOp.add)
                # evacuate log-odds (final sigmoid happens on host, exact);
                # last tile: DMA straight from par row 0, skipping the ACT
                if b0 == NBT - 1:
                    nc.sync.dma_start(out=d_out[0:1, bsl], in_=par[0:1, :])
                else:
                    nc.scalar.activation(out_sb[0:1, bsl], par[0:1, :],
                                         AF.Identity)

            nc.sync.dma_start(out=d_out[0:1, 0:(NBT - 1) * 512],
                              in_=out_sb[0:1, 0:(NBT - 1) * 512])

    nc.compile()
    return nc


def kernel(x, W, b, iw):
    x = np.asarray(x, np.float32)
    W = np.asarray(W, np.float32)
    b = np.asarray(b, np.float32)
    iw = np.asarray(iw, np.float32)

    # host prep: strictly upper-triangular U from iw (row-major i<j order),
    # pre-transposed operands so the contraction dim lands on SBUF partitions
    U = np.zeros((D, D), np.float32)
    iu, ju = np.triu_indices(D, k=1)
    U[iu, ju] = iw
    Wt = np.ascontiguousarray(W.T)          # lhsT for gates: Wt[k, dout]
    xT = x.T                                 # [D, B] view

    shared = {"Wtr": Wt, "Uf": U, "bias": b}
    in_maps = []
    for c in range(NCORES):
        m = dict(shared)
        m["xTf"] = np.ascontiguousarray(xT[:, c * BC:(c + 1) * BC])
        in_maps.append(m)

    if "nc" not in _CACHE:
        _CACHE["nc"] = _build()
    nc = _CACHE["nc"]

    res = run_bass_kernel_spmd(nc, in_maps, list(range(NCORES)))
    lo = np.concatenate(
        [res.results[c]["out"][0] for c in range(NCORES)]).astype(np.float64)
    out = 1.0 / (1.0 + np.exp(-np.clip(lo, -708.0, 708.0)))
    return out.reshape(B, 1).astype(np.float32)


# revision 11
# speedup vs baseline: 1.6609x; 1.4207x over previous
"""DINN forward kernel for Trainium2 (Bass/Tile), batch-sharded across 8 NeuronCores.

Reference computation (B=16384, D=512):
    gates  = sigmoid(x @ W.T + b)                       # [B, D]
    linear = sum(gates * x, axis=1)                     # [B]
    quad   = sum_{i<j} iw_ij * x_i * x_j                # [B]
    out    = sigmoid(linear + quad)[:, None]            # [B, 1]

Data-parallel sharding: x is split along the batch across the 8 cores;
W, b and the (strictly upper-triangular) interaction matrix U built from iw
are replicated. No collectives are needed in the forward pass.

Per-core kernel (batch shard of 2048 rows, 4 batch tiles of 512), all matmuls
in the "transposed" orientation with the contraction dim D on SBUF partitions
(host pre-transposes x -> xT).  Everything runs in f32r (~13-bit mantissa,
1 PE cycle/row vs 4 for exact fp32).  Per output chunk m (128 of the 512 d's):
    G^T[m,b] = sum_k Wt[k,m] xT[k,b]        4 matmuls -> psum pool pg
    T^T[m,b] = sum_{k<=m} U[k,m] xT[k,b]    m+1 matmuls -> psum pool ptp
                (U strictly upper -> 6/16 blocks skipped; the 10 nonzero
                 128x128 blocks are packed host-side, so only 0.625 MB DMA)
    sig_m = sigmoid(G^T + b_m)   ACT, per-partition bias
    s2_m  = sig_m + T^T_m        DVE   (frees the ptp psum bank)
    p_m   = s2_m * xT_m          Pool/GpSimd
    r    += ones^T @ p_m         PE matmul (f32r view) accumulating [1,512]
The ones-matmuls are deferred two m-groups so the PE never waits on the
ACT->DVE->Pool chain; each tile's [1,512] log-odds psum is DMA'd straight to
DRAM.  The final sigmoid is applied on host in float64 (exact, and the
kernel's output stays well-conditioned).

Precision: f32r carries ~13-bit mantissas (the f32r DMA itself rounds).
Quad log-odds error std ~ 1.2e-4 * sqrt(130816) ~ 0.04 on a +-360-scale
log-odds, which perturbs the final sigmoid only for the ~1% of rows near the
decision boundary: measured rel err ~1e-3, far under the 2e-2 gate.
x is DMA'd once as f32r and bitcast back to f32 for the element-wise ops.
"""
import sys

if "/opt/trn_rl_repo" not in sys.path:
    sys.path.insert(0, "/opt/trn_rl_repo")

import numpy as np

import concourse.tile as tile
from concourse import bacc, mybir
from concourse.bass_utils import run_bass_kernel_spmd

B, D = 16384, 512
NCORES = 8
BC = B // NCORES            # 2048 rows per core
NBT = BC // 512             # 4 batch tiles per core
NK = D // 128               # 4 contraction chunks

# packed strictly-upper-triangular U: nonzero 128x128 blocks (k <= m) in
# per-m usage order
UBLOCKS = [(m, k) for m in range(NK) for k in range(m + 1)]
UIDX = {mk: i for i, mk in enumerate(UBLOCKS)}
NUB = len(UBLOCKS)          # 10

f32 = mybir.dt.float32
f32r = mybir.dt.float32r
AF = mybir.ActivationFunctionType

_CACHE = {}


def _build():
    nc = bacc.Bacc("TRN2", target_bir_lowering=False, debug=False,
                   num_devices=NCORES)

    d_xTf = nc.declare_dram_parameter("xTf", [D, BC], f32, isOutput=False)
    d_Wtr = nc.declare_dram_parameter("Wtr", [D, D], f32r, isOutput=False)
    d_Upk = nc.declare_dram_parameter("Upk", [128, NUB * 128], f32r,
                                      isOutput=False)
    d_bias = nc.declare_dram_parameter("bias", [D], f32, isOutput=False)
    d_ones = nc.declare_dram_parameter("onesr", [128, 1], f32r, isOutput=False)
    d_out = nc.declare_dram_parameter("out", [1, BC], f32, isOutput=True)

    rearr = lambda ap: ap.rearrange("(c p) n -> p c n", p=128)

    with tile.TileContext(nc) as tc:
        with tc.tile_pool(name="const", bufs=1) as const, \
             tc.tile_pool(name="xin", bufs=2) as xin, \
             tc.tile_pool(name="elt", bufs=3) as elt, \
             tc.tile_pool(name="pg", bufs=3, space="PSUM") as pg, \
             tc.tile_pool(name="ptp", bufs=3, space="PSUM") as ptp, \
             tc.tile_pool(name="pop", bufs=2, space="PSUM") as pop:

            # ---- weights / constants (loaded once, replicated per core) ----
            Wtr_sb = const.tile([128, NK, D], f32r, tag="wtr")
            Upk_sb = const.tile([128, NUB, 128], f32r, tag="upk")
            bias_sb = const.tile([128, NK], f32, tag="bias")
            ones_sb = const.tile([128, 1], f32r, tag="ones")

            # f32r view of the fp32 dram tensor (the f32r DMA rounds, which
            # all consumers tolerate; see module docstring)
            xTr_r = rearr(d_xTf[:, :].bitcast(f32r))
            Wtr_d = rearr(d_Wtr[:, :])
            Upk_d = d_Upk[:, :].rearrange("p (i n) -> p i n", n=128)

            # first transfers in need order, gates k=0 pieces first, so the
            # PE starts ~2us in instead of waiting for whole tensors
            xr0 = xin.tile([128, NK, 512], f32r, tag="xr")
            nc.sync.dma_start(out=xr0[:, 0, :], in_=xTr_r[:, 0, 0:512])
            nc.sync.dma_start(out=Wtr_sb[:, 0, :], in_=Wtr_d[:, 0, :])
            nc.sync.dma_start(out=xr0[:, 1:NK, :], in_=xTr_r[:, 1:NK, 0:512])
            nc.sync.dma_start(out=Wtr_sb[:, 1:NK, :], in_=Wtr_d[:, 1:NK, :])
            nc.sync.dma_start(out=Upk_sb[:, 0:3, :], in_=Upk_d[:, 0:3, :])
            nc.sync.dma_start(
                out=bias_sb, in_=d_bias[:].rearrange("(c p) -> p c", p=128))
            nc.sync.dma_start(out=ones_sb, in_=d_ones[:, :])
            nc.sync.dma_start(out=Upk_sb[:, 3:NUB, :], in_=Upk_d[:, 3:NUB, :])

            # ones-matmuls deferred >= 2 m-groups behind the producing group
            # so the PE never waits on the ACT->DVE->Pool chain
            pending = []        # (group_idx, emit_fn)

            def flush(now):
                while pending and pending[0][0] <= now - 2:
                    pending.pop(0)[1]()

            for b0 in range(NBT):
                bsl = slice(b0 * 512, (b0 + 1) * 512)
                if b0 == 0:
                    xr = xr0
                else:
                    xr = xin.tile([128, NK, 512], f32r, tag="xr")
                    nc.sync.dma_start(out=xr, in_=xTr_r[:, :, bsl])
                xf = xr[:, :, :].bitcast(f32)

                po = pop.tile([1, 512], f32, tag="po")

                for m in range(NK):
                    g = b0 * NK + m
                    # gates: G^T chunk
                    psum_g = pg.tile([128, 512], f32, tag="pg")
                    msl = slice(m * 128, (m + 1) * 128)
                    for k in range(NK):
                        nc.tensor.matmul(
                            psum_g, Wtr_sb[:, k, msl], xr[:, k, :],
                            start=(k == 0), stop=(k == NK - 1))
                    # quad: T^T chunk from the packed nonzero blocks
                    ptm = ptp.tile([128, 512], f32, tag="pt")
                    for k in range(m + 1):
                        nc.tensor.matmul(
                            ptm, Upk_sb[:, UIDX[(m, k)], :], xr[:, k, :],
                            start=(k == 0), stop=(k == m))
                    flush(g)

                    sig = elt.tile([128, 512], f32, tag="sig")
                    nc.scalar.activation(sig, psum_g, AF.Sigmoid,
                                         bias=bias_sb[:, m:m + 1], scale=1.0)
                    s2 = elt.tile([128, 512], f32, tag="s2")
                    nc.vector.tensor_add(s2, sig, ptm)
                    # the mul writes f32r directly (engine rounds on write):
                    # required by the BIR verifier for the f32r ones-matmul
                    p_m = elt.tile([128, 512], f32r, tag="p")
                    nc.gpsimd.tensor_mul(p_m, s2, xf[:, m, :])

                    def emit(po=po, p_m=p_m, m=m, bsl=bsl):
                        nc.tensor.matmul(
                            po, ones_sb, p_m[:, :],
                            start=(m == 0), stop=(m == NK - 1))
                        if m == NK - 1:
                            # tile finished: [1, 512] log-odds PSUM -> SBUF
                            # (ACT copy) -> DMA to DRAM.  Emitted HERE, after
                            # the stop matmul, so the emission-order dependency
                            # tracking sees all 4 accumulating matmuls.
                            ot = elt.tile([1, 512], f32, tag="ot")
                            nc.scalar.activation(ot, po[0:1, :], AF.Copy)
                            nc.sync.dma_start(out=d_out[0:1, bsl], in_=ot)
                    pending.append((g, emit))

                if b0 == NBT - 1:
                    while pending:
                        pending.pop(0)[1]()

    nc.compile()
    return nc


def kernel(x, W, b, iw):
    x = np.asarray(x, np.float32)
    W = np.asarray(W, np.float32)
    b = np.asarray(b, np.float32)
    iw = np.asarray(iw, np.float32)

    # host prep: strictly upper-triangular U from iw (row-major i<j order),
    # pre-transposed operands so the contraction dim lands on SBUF partitions
    U = np.zeros((D, D), np.float32)
    iu, ju = np.triu_indices(D, k=1)
    U[iu, ju] = iw
    Wt = np.ascontiguousarray(W.T)          # lhsT for gates: Wt[k, dout]
    # pack the 10 nonzero 128x128 blocks of U, contraction dim on partitions
    Upk = np.empty((128, NUB * 128), np.float32)
    for i, (m, k) in enumerate(UBLOCKS):
        Upk[:, i * 128:(i + 1) * 128] = U[k * 128:(k + 1) * 128,
                                          m * 128:(m + 1) * 128]
    xT = x.T                                 # [D, B] view

    shared = {"Wtr": Wt, "Upk": Upk, "bias": b,
              "onesr": np.ones((128, 1), np.float32)}
    in_maps = []
    for c in range(NCORES):
        mp = dict(shared)
        mp["xTf"] = np.ascontiguousarray(xT[:, c * BC:(c + 1) * BC])
        in_maps.append(mp)

    if "nc" not in _CACHE:
        _CACHE["nc"] = _build()
    nc = _CACHE["nc"]

    res = run_bass_kernel_spmd(nc, in_maps, list(range(NCORES)))
    lo = np.concatenate(
        [res.results[c]["out"][0] for c in range(NCORES)]).astype(np.float64)
    out = 1.0 / (1.0 + np.exp(-np.clip(lo, -708.0, 708.0)))
    return out.reshape(B, 1).astype(np.float32)


# revision 12
# speedup vs baseline: 1.8751x; 1.1289x over previous
"""DINN forward kernel for Trainium2 (Bass/Tile), batch-sharded across 8 NeuronCores.

Reference computation (B=16384, D=512):
    gates  = sigmoid(x @ W.T + b)                       # [B, D]
    linear = sum(gates * x, axis=1)                     # [B]
    quad   = sum_{i<j} iw_ij * x_i * x_j                # [B]
    out    = sigmoid(linear + quad)[:, None]            # [B, 1]

Data-parallel sharding: x is split along the batch across the 8 cores;
W, b and the (strictly upper-triangular) interaction matrix U built from iw
are replicated. No collectives are needed in the forward pass.

Per-core kernel (batch shard of 2048 rows, 4 batch tiles of 512), all matmuls
in the "transposed" orientation with the contraction dim D on SBUF partitions
(host pre-transposes x -> xT).  Per output chunk m (128 of the 512 d's):
    G^T[m,b] = sum_k Wt[k,m] xT[k,b]     2 fp8 DoubleRow matmuls (0.5 cy/row;
                                          W is pre-scaled by 8 on the host to
                                          dodge fp8 subnormals, undone by the
                                          ACT scale)
    T^T[m,b] = sum_{k<=m} U[k,m] xT[k,b]  m+1 f32r matmuls (1 cy/row, ~13-bit
                                          mantissa; exact-enough, see below).
                 U strictly upper -> 6/16 blocks skipped; the 10 nonzero
                 128x128 blocks are packed host-side (0.625 MB DMA)
    sig_m = sigmoid(G^T/8 + b_m)  ACT, per-partition bias + scalar scale
    s2_m  = sig_m + T^T_m         DVE   (frees the quad psum bank)
    p_m   = s2_m * xT_m           Pool/GpSimd, written as f32r
    r    += ones^T @ p_m          PE matmul accumulating [1,512] in PSUM
The ones-matmuls (and each tile's ACT drain + DMA) are deferred three
m-groups so the PE never waits on the ACT->DVE->Pool chain.  The kernel
returns log-odds; the final sigmoid runs on host in float64.

Precision: the quad term needs ~13-bit products (f32r): its log-odds error
std is ~1.2e-4 * sqrt(130816) ~ 0.04 on a +-360-scale log-odds.  The gates
tolerate fp8 (z-error ~0.1 -> linear-part error ~0.4), perturbing the final
sigmoid only for the ~1% of rows near the decision boundary: measured rel
err stays well under the 2e-2 gate.  x is DMA'd once as f32r (quad matmuls +
element-wise) and once as fp8 (gates).
"""
import sys

if "/opt/trn_rl_repo" not in sys.path:
    sys.path.insert(0, "/opt/trn_rl_repo")

import numpy as np
from ml_dtypes import float8_e4m3

import concourse.tile as tile
from concourse import bacc, mybir
from concourse.bass_utils import run_bass_kernel_spmd

B, D = 16384, 512
NCORES = 8
BC = B // NCORES            # 2048 rows per core
NBT = BC // 512             # 4 batch tiles per core
NK = D // 128               # 4 contraction chunks

# packed strictly-upper-triangular U: nonzero 128x128 blocks (k <= m) in
# per-m usage order
UBLOCKS = [(m, k) for m in range(NK) for k in range(m + 1)]
UIDX = {mk: i for i, mk in enumerate(UBLOCKS)}
NUB = len(UBLOCKS)          # 10

WSCALE = 8.0                # host pre-scale of W for fp8 range

f32 = mybir.dt.float32
f32r = mybir.dt.float32r
f8 = mybir.dt.float8e4
AF = mybir.ActivationFunctionType
DR = mybir.MatmulPerfMode.DoubleRow

_CACHE = {}


def _build():
    nc = bacc.Bacc("TRN2", target_bir_lowering=False, debug=False,
                   num_devices=NCORES)

    d_xTf = nc.declare_dram_parameter("xTf", [D, BC], f32, isOutput=False)
    d_xT8 = nc.declare_dram_parameter("xT8", [D, BC], f8, isOutput=False)
    d_W8 = nc.declare_dram_parameter("W8", [D, D], f8, isOutput=False)
    d_Upk = nc.declare_dram_parameter("Upk", [128, NUB * 128], f32r,
                                      isOutput=False)
    d_bias = nc.declare_dram_parameter("bias", [D], f32, isOutput=False)
    d_ones = nc.declare_dram_parameter("onesr", [128, 1], f32r, isOutput=False)
    d_out = nc.declare_dram_parameter("out", [1, BC], f32, isOutput=True)

    rearr = lambda ap: ap.rearrange("(c p) n -> p c n", p=128)

    with tile.TileContext(nc) as tc:
        with tc.tile_pool(name="const", bufs=1) as const, \
             tc.tile_pool(name="xin", bufs=2) as xin, \
             tc.tile_pool(name="elt", bufs=3) as elt, \
             tc.tile_pool(name="pel", bufs=4) as pel, \
             tc.tile_pool(name="pg", bufs=3, space="PSUM") as pg, \
             tc.tile_pool(name="ptp", bufs=3, space="PSUM") as ptp, \
             tc.tile_pool(name="pop", bufs=2, space="PSUM") as pop:

            # ---- weights / constants (loaded once, replicated per core) ----
            W8_sb = const.tile([128, NK, D], f8, tag="w8")
            Upk_sb = const.tile([128, NUB, 128], f32r, tag="upk")
            bias_sb = const.tile([128, NK], f32, tag="bias")
            ones_sb = const.tile([128, 1], f32r, tag="ones")

            xTr_r = rearr(d_xTf[:, :].bitcast(f32r))
            xT8_r = rearr(d_xT8[:, :])
            W8_d = rearr(d_W8[:, :])
            Upk_d = d_Upk[:, :].rearrange("p (i n) -> p i n", n=128)

            # prologue transfers in need order: the fp8 gates operands are
            # tiny (0.25 MB each) so the PE starts ~2us in; the f32r x tile
            # and packed U stream in while the gates run
            x80 = xin.tile([128, NK, 512], f8, tag="x8")
            nc.sync.dma_start(out=x80, in_=xT8_r[:, :, 0:512])
            nc.sync.dma_start(out=W8_sb, in_=W8_d)
            nc.sync.dma_start(
                out=bias_sb, in_=d_bias[:].rearrange("(c p) -> p c", p=128))
            nc.sync.dma_start(out=ones_sb, in_=d_ones[:, :])
            xr0 = xin.tile([128, NK, 512], f32r, tag="xr")
            nc.sync.dma_start(out=Upk_sb[:, 0:3, :], in_=Upk_d[:, 0:3, :])
            nc.sync.dma_start(out=xr0[:, 0:2, :], in_=xTr_r[:, 0:2, 0:512])
            nc.sync.dma_start(out=Upk_sb[:, 3:6, :], in_=Upk_d[:, 3:6, :])
            nc.sync.dma_start(out=xr0[:, 2:NK, :], in_=xTr_r[:, 2:NK, 0:512])
            nc.sync.dma_start(out=Upk_sb[:, 6:NUB, :], in_=Upk_d[:, 6:NUB, :])

            # ones-matmuls deferred >= 3 m-groups behind the producing group
            # so the PE never waits on the ACT->DVE->Pool chain
            pending = []        # (group_idx, emit_fn)

            def flush(now):
                while pending and pending[0][0] <= now - 3:
                    pending.pop(0)[1]()

            for b0 in range(NBT):
                bsl = slice(b0 * 512, (b0 + 1) * 512)
                if b0 == 0:
                    x8, xr = x80, xr0
                else:
                    x8 = xin.tile([128, NK, 512], f8, tag="x8")
                    nc.sync.dma_start(out=x8, in_=xT8_r[:, :, bsl])
                    xr = xin.tile([128, NK, 512], f32r, tag="xr")
                    nc.sync.dma_start(out=xr, in_=xTr_r[:, :, bsl])
                xf = xr[:, :, :].bitcast(f32)

                po = pop.tile([1, 512], f32, tag="po")

                for m in range(NK):
                    g = b0 * NK + m
                    msl = slice(m * 128, (m + 1) * 128)
                    # gates: G^T chunk, 2 fp8 DoubleRow matmuls (K=256 each)
                    psum_g = pg.tile([128, 512], f32, tag="pg")
                    for kc in range(2):
                        nc.tensor.matmul(
                            psum_g, W8_sb[:, 2 * kc:2 * kc + 2, msl],
                            x8[:, 2 * kc:2 * kc + 2, :],
                            start=(kc == 0), stop=(kc == 1), perf_mode=DR)
                    # quad: T^T chunk from the packed nonzero f32r blocks
                    ptm = ptp.tile([128, 512], f32, tag="pt")
                    for k in range(m + 1):
                        nc.tensor.matmul(
                            ptm, Upk_sb[:, UIDX[(m, k)], :], xr[:, k, :],
                            start=(k == 0), stop=(k == m))
                    flush(g)

                    sig = elt.tile([128, 512], f32, tag="sig")
                    nc.scalar.activation(sig, psum_g, AF.Sigmoid,
                                         bias=bias_sb[:, m:m + 1],
                                         scale=1.0 / WSCALE)
                    s2 = elt.tile([128, 512], f32, tag="s2")
                    nc.vector.tensor_add(s2, sig, ptm)
                    # the mul writes f32r directly (engine rounds on write):
                    # required by the BIR verifier for the f32r ones-matmul
                    p_m = pel.tile([128, 512], f32r, tag="p")
                    nc.gpsimd.tensor_mul(p_m, s2, xf[:, m, :])

                    def emit(po=po, p_m=p_m, m=m, bsl=bsl):
                        nc.tensor.matmul(
                            po, ones_sb, p_m[:, :],
                            start=(m == 0), stop=(m == NK - 1))
                        if m == NK - 1:
                            # tile finished: [1, 512] log-odds PSUM -> SBUF
                            # (ACT copy) -> DMA to DRAM.  Emitted HERE, after
                            # the stop matmul, so the emission-order dependency
                            # tracking sees all 4 accumulating matmuls.
                            ot = elt.tile([1, 512], f32, tag="ot")
                            nc.scalar.activation(ot, po[0:1, :], AF.Copy)
                            nc.sync.dma_start(out=d_out[0:1, bsl], in_=ot)
                    pending.append((g, emit))

                if b0 == NBT - 1:
                    while pending:
                        pending.pop(0)[1]()

    nc.compile()
    return nc


def kernel(x, W, b, iw):
    x = np.asarray(x, np.float32)
    W = np.asarray(W, np.float32)
    b = np.asarray(b, np.float32)
    iw = np.asarray(iw, np.float32)

    # host prep: strictly upper-triangular U from iw (row-major i<j order),
    # pre-transposed operands so the contraction dim lands on SBUF partitions
    U = np.zeros((D, D), np.float32)
    iu, ju = np.triu_indices(D, k=1)
    U[iu, ju] = iw
    # pack the 10 nonzero 128x128 blocks of U, contraction dim on partitions
    Upk = np.empty((128, NUB * 128), np.float32)
    for i, (m, k) in enumerate(UBLOCKS):
        Upk[:, i * 128:(i + 1) * 128] = U[k * 128:(k + 1) * 128,
                                          m * 128:(m + 1) * 128]
    W8 = np.ascontiguousarray(W.T * WSCALE).astype(float8_e4m3)
    xT = x.T                                 # [D, B] view
    xT8 = xT.astype(float8_e4m3)

    shared = {"W8": W8, "Upk": Upk, "bias": b,
              "onesr": np.ones((128, 1), np.float32)}
    in_maps = []
    for c in range(NCORES):
        mp = dict(shared)
        mp["xTf"] = np.ascontiguousarray(xT[:, c * BC:(c + 1) * BC])
        mp["xT8"] = np.ascontiguousarray(xT8[:, c * BC:(c + 1) * BC])
        in_maps.append(mp)

    if "nc" not in _CACHE:
        _CACHE["nc"] = _build()
    nc = _CACHE["nc"]

    res = run_bass_kernel_spmd(nc, in_maps, list(range(NCORES)))
    lo = np.concatenate(
        [res.results[c]["out"][0] for c in range(NCORES)]).astype(np.float64)
    out = 1.0 / (1.0 + np.exp(-np.clip(lo, -708.0, 708.0)))
    return out.reshape(B, 1).astype(np.float32)


# revision 13
# speedup vs baseline: 2.3166x; 1.2355x over previous
"""DINN forward kernel for Trainium2 (Bass/Tile), batch-sharded across 8 NeuronCores.

Reference computation (B=16384, D=512):
    gates  = sigmoid(x @ W.T + b)                       # [B, D]
    linear = sum(gates * x, axis=1)                     # [B]
    quad   = sum_{i<j} iw_ij * x_i * x_j                # [B]
    out    = sigmoid(linear + quad)[:, None]            # [B, 1]

Data-parallel sharding: x is split along the batch across the 8 cores;
W, b and the (strictly upper-triangular) interaction matrix U built from iw
are replicated. No collectives are needed in the forward pass.

Per-core kernel (batch shard of 2048 rows, 4 batch tiles of 512), all matmuls
in the "transposed" orientation with the contraction dim D on SBUF partitions
(host pre-transposes x -> xT).  Per output chunk m (128 of the 512 d's):
    G^T[m,b] = sum_k Wt[k,m] xT[k,b]     2 fp8 DoubleRow matmuls (0.5 cy/row;
                                          W is pre-scaled by 8 on the host to
                                          dodge fp8 subnormals, undone by the
                                          ACT scale)
    T^T[m,b] = sum_{k<=m} U[k,m] xT[k,b]  m+1 fp16 matmuls (1 cy/row).
                 U strictly upper -> 6/16 blocks skipped; the 10 nonzero
                 128x128 blocks are packed host-side (0.3 MB DMA)
    sig_m = sigmoid(G^T/8 + b_m)  ACT, psum -> fp16 sbuf
    s2_m  = sig_m + T^T_m         DVE, (fp16 + f32 psum) -> fp16
    p_m   = s2_m * xT_m           DVE all-fp16 (4x mode, ~194 ns)
    r    += ones^T @ p_m          PE fp16 matmul accumulating [1,512] in PSUM
The ones-matmuls (and each tile's ACT drain + DMA) are deferred three
m-groups so the PE never waits on the ACT->DVE chain.  The kernel returns
log-odds; the final sigmoid runs on host in float64.

Precision budget (log-odds scale ~N(0, 362), tolerance rel 2e-2): fp8 gates
give linear-part error ~0.4; fp16 x/U give quad error ~0.15; both only
perturb the ~1% of rows near the decision boundary.  Measured rel err ~4e-3,
5x under the gate.  All inputs are DMA'd once: x as fp16 + fp8, W as fp8,
U packed fp16 -> 3.6 MB per core total.
"""
import sys

if "/opt/trn_rl_repo" not in sys.path:
    sys.path.insert(0, "/opt/trn_rl_repo")

import numpy as np
from ml_dtypes import float8_e4m3

import concourse.tile as tile
from concourse import bacc, mybir
from concourse.bass_utils import run_bass_kernel_spmd

B, D = 16384, 512
NCORES = 8
BC = B // NCORES            # 2048 rows per core
NBT = BC // 512             # 4 batch tiles per core
NK = D // 128               # 4 contraction chunks

# packed strictly-upper-triangular U: nonzero 128x128 blocks (k <= m) in
# per-m usage order
UBLOCKS = [(m, k) for m in range(NK) for k in range(m + 1)]
UIDX = {mk: i for i, mk in enumerate(UBLOCKS)}
NUB = len(UBLOCKS)          # 10

WSCALE = 8.0                # host pre-scale of W for fp8 range

f32 = mybir.dt.float32
f16 = mybir.dt.float16
f8 = mybir.dt.float8e4
AF = mybir.ActivationFunctionType
DR = mybir.MatmulPerfMode.DoubleRow

_CACHE = {}


def _build():
    nc = bacc.Bacc("TRN2", target_bir_lowering=False, debug=False,
                   num_devices=NCORES)

    d_x16 = nc.declare_dram_parameter("x16", [D, BC], f16, isOutput=False)
    d_xT8 = nc.declare_dram_parameter("xT8", [D, BC], f8, isOutput=False)
    d_W8 = nc.declare_dram_parameter("W8", [D, D], f8, isOutput=False)
    d_Upk = nc.declare_dram_parameter("Upk", [128, NUB * 128], f16,
                                      isOutput=False)
    d_bias = nc.declare_dram_parameter("bias", [D], f32, isOutput=False)
    d_ones = nc.declare_dram_parameter("ones16", [128, 1], f16, isOutput=False)
    d_out = nc.declare_dram_parameter("out", [1, BC], f32, isOutput=True)

    rearr = lambda ap: ap.rearrange("(c p) n -> p c n", p=128)

    with tile.TileContext(nc) as tc:
        with tc.tile_pool(name="const", bufs=1) as const, \
             tc.tile_pool(name="xin", bufs=2) as xin, \
             tc.tile_pool(name="elt", bufs=3) as elt, \
             tc.tile_pool(name="pel", bufs=4) as pel, \
             tc.tile_pool(name="pg", bufs=3, space="PSUM") as pg, \
             tc.tile_pool(name="ptp", bufs=3, space="PSUM") as ptp, \
             tc.tile_pool(name="pop", bufs=2, space="PSUM") as pop:

            # ---- weights / constants (loaded once, replicated per core) ----
            W8_sb = const.tile([128, NK, D], f8, tag="w8")
            Upk_sb = const.tile([128, NUB, 128], f16, tag="upk")
            bias_sb = const.tile([128, NK], f32, tag="bias")
            ones_sb = const.tile([128, 1], f16, tag="ones")

            x16_r = rearr(d_x16[:, :])
            xT8_r = rearr(d_xT8[:, :])
            W8_d = rearr(d_W8[:, :])
            Upk_d = d_Upk[:, :].rearrange("p (i n) -> p i n", n=128)

            # prologue transfers in need order: the fp8 gates operands are
            # tiny (0.25 MB each) so the PE starts ~2us in; the fp16 x tile
            # and packed U stream in while the gates run
            nc.sync.dma_start(out=ones_sb, in_=d_ones[:, :])
            x80 = xin.tile([128, NK, 512], f8, tag="x8")
            nc.sync.dma_start(out=x80, in_=xT8_r[:, :, 0:512])
            nc.sync.dma_start(out=W8_sb, in_=W8_d)
            nc.sync.dma_start(
                out=bias_sb, in_=d_bias[:].rearrange("(c p) -> p c", p=128))
            x160 = xin.tile([128, NK, 512], f16, tag="x16")
            nc.sync.dma_start(out=Upk_sb[:, 0:3, :], in_=Upk_d[:, 0:3, :])
            nc.sync.dma_start(out=x160[:, 0:2, :], in_=x16_r[:, 0:2, 0:512])
            nc.sync.dma_start(out=Upk_sb[:, 3:6, :], in_=Upk_d[:, 3:6, :])
            nc.sync.dma_start(out=x160[:, 2:NK, :], in_=x16_r[:, 2:NK, 0:512])
            nc.sync.dma_start(out=Upk_sb[:, 6:NUB, :], in_=Upk_d[:, 6:NUB, :])

            # ones-matmuls deferred >= 3 m-groups behind the producing group
            # so the PE never waits on the ACT->DVE chain
            pending = []        # (group_idx, emit_fn)

            def flush(now):
                while pending and pending[0][0] <= now - 3:
                    pending.pop(0)[1]()

            for b0 in range(NBT):
                bsl = slice(b0 * 512, (b0 + 1) * 512)
                if b0 == 0:
                    x8, x16 = x80, x160
                else:
                    x8 = xin.tile([128, NK, 512], f8, tag="x8")
                    nc.sync.dma_start(out=x8, in_=xT8_r[:, :, bsl])
                    x16 = xin.tile([128, NK, 512], f16, tag="x16")
                    nc.sync.dma_start(out=x16, in_=x16_r[:, :, bsl])

                po = pop.tile([1, 512], f32, tag="po")

                for m in range(NK):
                    g = b0 * NK + m
                    msl = slice(m * 128, (m + 1) * 128)
                    # gates: G^T chunk, 2 fp8 DoubleRow matmuls (K=256 each)
                    psum_g = pg.tile([128, 512], f32, tag="pg")
                    for kc in range(2):
                        nc.tensor.matmul(
                            psum_g, W8_sb[:, 2 * kc:2 * kc + 2, msl],
                            x8[:, 2 * kc:2 * kc + 2, :],
                            start=(kc == 0), stop=(kc == 1), perf_mode=DR)
                    # quad: T^T chunk from the packed nonzero fp16 blocks
                    ptm = ptp.tile([128, 512], f32, tag="pt")
                    for k in range(m + 1):
                        nc.tensor.matmul(
                            ptm, Upk_sb[:, UIDX[(m, k)], :], x16[:, k, :],
                            start=(k == 0), stop=(k == m))
                    flush(g)

                    sig = elt.tile([128, 512], f16, tag="sig")
                    nc.scalar.activation(sig, psum_g, AF.Sigmoid,
                                         bias=bias_sb[:, m:m + 1],
                                         scale=1.0 / WSCALE)
                    s2 = elt.tile([128, 512], f16, tag="s2")
                    nc.vector.tensor_add(s2, sig, ptm)
                    p_m = pel.tile([128, 512], f16, tag="p")
                    nc.vector.tensor_mul(p_m, s2, x16[:, m, :])

                    def emit(po=po, p_m=p_m, m=m, bsl=bsl):
                        nc.tensor.matmul(
                            po, ones_sb, p_m[:, :],
                            start=(m == 0), stop=(m == NK - 1))
                        if m == NK - 1:
                            # tile finished: [1, 512] log-odds PSUM -> SBUF
                            # (ACT copy) -> DMA to DRAM.  Emitted HERE, after
                            # the stop matmul, so the emission-order dependency
                            # tracking sees all 4 accumulating matmuls.
                            ot = elt.tile([1, 512], f32, tag="ot")
                            nc.scalar.activation(ot, po[0:1, :], AF.Copy)
                            nc.sync.dma_start(out=d_out[0:1, bsl], in_=ot)
                    pending.append((g, emit))

                if b0 == NBT - 1:
                    while pending:
                        pending.pop(0)[1]()

    nc.compile()
    return nc


def kernel(x, W, b, iw):
    x = np.asarray(x, np.float32)
    W = np.asarray(W, np.float32)
    b = np.asarray(b, np.float32)
    iw = np.asarray(iw, np.float32)

    # host prep: strictly upper-triangular U from iw (row-major i<j order),
    # pre-transposed operands so the contraction dim lands on SBUF partitions
    U = np.zeros((D, D), np.float32)
    iu, ju = np.triu_indices(D, k=1)
    U[iu, ju] = iw
    # pack the 10 nonzero 128x128 blocks of U, contraction dim on partitions
    Upk = np.empty((128, NUB * 128), np.float16)
    for i, (m, k) in enumerate(UBLOCKS):
        Upk[:, i * 128:(i + 1) * 128] = U[k * 128:(k + 1) * 128,
                                          m * 128:(m + 1) * 128]
    W8 = np.ascontiguousarray(W.T * WSCALE).astype(float8_e4m3)
    xT = x.T                                 # [D, B] view
    xT8 = xT.astype(float8_e4m3)
    xT16 = xT.astype(np.float16)

    shared = {"W8": W8, "Upk": Upk, "bias": b,
              "ones16": np.ones((128, 1), np.float16)}
    in_maps = []
    for c in range(NCORES):
        mp = dict(shared)
        mp["x16"] = np.ascontiguousarray(xT16[:, c * BC:(c + 1) * BC])
        mp["xT8"] = np.ascontiguousarray(xT8[:, c * BC:(c + 1) * BC])
        in_maps.append(mp)

    if "nc" not in _CACHE:
        _CACHE["nc"] = _build()
    nc = _CACHE["nc"]

    res = run_bass_kernel_spmd(nc, in_maps, list(range(NCORES)))
    lo = np.concatenate(
        [res.results[c]["out"][0] for c in range(NCORES)]).astype(np.float64)
    out = 1.0 / (1.0 + np.exp(-np.clip(lo, -708.0, 708.0)))
    return out.reshape(B, 1).astype(np.float32)
